# revision 4
# baseline (speedup 1.0000x reference)
"""Grouped GRU (G=8, H=I=64, B=32, T=1000) Trainium2 Bass kernel.

Sharding: one group per NeuronCore (8 groups / 8 cores), expert-style.
Each core runs the full T=1000 recurrence for its group, batch 32 in the
free dimension, hidden dim on partitions.

Per-core layout (everything fp32):
  xT       [65, T*B]  : x for this group, transposed to [I, T, B] (+ ones row)
  w_xp_rz  [65, 128]  : lhsT for xp_{r,z} = Wih_rz @ x + bih_rz (bias row folded)
  w_xp_n   [65, 64]   : lhsT for xp_n = Wih_n @ x + bih_n
  w_hh_rz  [64, 128]  : lhsT for U_rz @ h
  w_hh_n   [64, 64]   : lhsT for U_n @ h
  bhh_rz   [128, 1]   : added inside the sigmoid via ACT bias
  bhh_n    [1, 64]    : outer-product prefilled into the ghn psum bank
  h0T      [64, 32]   : initial hidden (transposed)
  y        [64, T*B]  : output, [H, T, B]

Recurrence per step (columns c = 32 per step inside a 16-step psum chunk):
  psum_rz[:,c] = xp_rz (prefill) + U_rz@h          (PE accumulate)
  psum_ghn[:,c] = bhh_n (prefill) + U_n@h          (PE accumulate)
  rz = sigmoid(psum_rz + bhh_rz)                   (ACT, bias fold)
  v1 = r * psum_ghn ; v2 = v1 + xpn_sb             (DVE)
  n  = tanh(v2)                                    (ACT)
  d = h - n ; dz = d*z ; h' = dz + n  -> ychunk    (DVE)
"""

import sys

if "/opt/trn_rl_repo" not in sys.path:
    sys.path.insert(0, "/opt/trn_rl_repo")

from contextlib import ExitStack

import numpy as np

import concourse.bass as bass
import concourse.tile as tile
from concourse import bacc, mybir
from concourse.bass_utils import run_bass_kernel_spmd

G, H, I, B, T = 8, 64, 64, 32, 1000
IP1 = I + 1
F32 = mybir.dt.float32
AF = mybir.ActivationFunctionType

CH = 16  # timesteps per psum chunk (16*32 = 512 cols = one bank)


def _chunks():
    out, t0 = [], 0
    while t0 < T:
        s = min(CH, T - t0)
        out.append((t0, s))
        t0 += s
    return out


def _build_program():
    nc = bacc.Bacc(
        "TRN2", target_bir_lowering=False, debug=False, num_devices=G
    )
    xT = nc.dram_tensor("xT", [IP1, T * B], F32, kind="ExternalInput").ap()
    w_xp_rz = nc.dram_tensor("w_xp_rz", [IP1, 2 * H], F32, kind="ExternalInput").ap()
    w_xp_n = nc.dram_tensor("w_xp_n", [IP1, H], F32, kind="ExternalInput").ap()
    w_hh_rz = nc.dram_tensor("w_hh_rz", [H, 2 * H], F32, kind="ExternalInput").ap()
    w_hh_n = nc.dram_tensor("w_hh_n", [H, H], F32, kind="ExternalInput").ap()
    bhh_r = nc.dram_tensor("bhh_r", [H, 1], F32, kind="ExternalInput").ap()
    bhh_z = nc.dram_tensor("bhh_z", [H, 1], F32, kind="ExternalInput").ap()
    bhh_n = nc.dram_tensor("bhh_n", [1, H], F32, kind="ExternalInput").ap()
    h0T = nc.dram_tensor("h0T", [H, B], F32, kind="ExternalInput").ap()
    y = nc.dram_tensor("y", [H, T * B], F32, kind="ExternalOutput").ap()

    with tile.TileContext(nc) as tc, ExitStack() as ctx:
        const = ctx.enter_context(tc.tile_pool(name="const", bufs=1))
        xpool = ctx.enter_context(tc.tile_pool(name="xp", bufs=3))
        ypool = ctx.enter_context(tc.tile_pool(name="yp", bufs=3))
        xpnpool = ctx.enter_context(tc.tile_pool(name="xpnp", bufs=2))
        psum = ctx.enter_context(tc.tile_pool(name="ps", bufs=2, space="PSUM"))
        steps = ctx.enter_context(tc.tile_pool(name="st", bufs=4))

        def const_tile(ap, shape, tag):
            t = const.tile(shape, F32, tag=tag)
            nc.sync.dma_start(t[:], ap)
            return t

        w_xp_rz_t = const_tile(w_xp_rz, [IP1, 2 * H], "w_xp_rz")
        w_xp_n_t = const_tile(w_xp_n, [IP1, H], "w_xp_n")
        w_hh_rz_t = const_tile(w_hh_rz, [H, 2 * H], "w_hh_rz")
        w_hh_n_t = const_tile(w_hh_n, [H, H], "w_hh_n")
        bhh_r_t = const_tile(bhh_r, [H, 1], "bhh_r")
        bhh_z_t = const_tile(bhh_z, [H, 1], "bhh_z")
        bhh_n_t = const_tile(bhh_n, [1, H], "bhh_n")
        h0_t = const_tile(h0T, [H, B], "h0")
        ones_t = const.tile([1, CH * B], F32, tag="ones")
        nc.vector.memset(ones_t[:], 1.0)

        hprev = h0_t[:, :]
        for t0, S in _chunks():
            N = B * S
            cbase = t0 * B
            xch = xpool.tile([IP1, CH * B], F32, tag="xch")
            nc.sync.dma_start(xch[:, :N], xT[:, cbase : cbase + N])

            prz = psum.tile([2 * H, CH * B], F32, tag="prz")
            pghn = psum.tile([H, CH * B], F32, tag="pghn")
            pxpn = psum.tile([H, CH * B], F32, tag="pxpn")
            nc.tensor.matmul(prz[:, :N], w_xp_rz_t[:], xch[:, :N], start=True, stop=False)
            nc.tensor.matmul(
                pghn[:, :N], bhh_n_t[:], ones_t[:, :N], start=True, stop=False
            )
            nc.tensor.matmul(pxpn[:, :N], w_xp_n_t[:], xch[:, :N], start=True, stop=True)
            xpn_sb = xpnpool.tile([H, CH * B], F32, tag="xpn_sb")
            nc.scalar.copy(xpn_sb[:, :N], pxpn[:, :N])

            ych = ypool.tile([H, CH * B], F32, tag="ych")
            for k in range(S):
                c = slice(B * k, B * k + B)
                nc.tensor.matmul(prz[:, c], w_hh_rz_t[:], hprev, start=False, stop=True)
                nc.tensor.matmul(pghn[:, c], w_hh_n_t[:], hprev, start=False, stop=True)
                r_sb = steps.tile([H, B], F32, tag="r_sb")
                nc.scalar.activation(r_sb[:], prz[0:H, c], AF.Sigmoid, bias=bhh_r_t[:])
                z_sb = steps.tile([H, B], F32, tag="z_sb")
                nc.scalar.activation(z_sb[:], prz[H : 2 * H, c], AF.Sigmoid, bias=bhh_z_t[:])
                v1 = steps.tile([H, B], F32, tag="v1")
                nc.vector.tensor_mul(v1[:], r_sb[:], pghn[:, c])
                v2 = steps.tile([H, B], F32, tag="v2")
                nc.vector.tensor_add(v2[:], v1[:], xpn_sb[:, c])
                n_t = steps.tile([H, B], F32, tag="n")
                nc.scalar.activation(n_t[:], v2[:], AF.Tanh)
                d = steps.tile([H, B], F32, tag="d")
                nc.vector.tensor_sub(d[:], hprev, n_t[:])
                dz = steps.tile([H, B], F32, tag="dz")
                nc.vector.tensor_mul(dz[:], d[:], z_sb[:])
                nc.vector.tensor_add(ych[:, c], dz[:], n_t[:])
                hprev = ych[:, c]
            nc.sync.dma_start(y[:, cbase : cbase + N], ych[:, :N])

    nc.compile()
    return nc


def _prep_inputs(x, h0, Wih, Whh, bih, bhh):
    """Host-side shard + layout prep. Returns per-core in_maps."""
    x = np.ascontiguousarray(x, dtype=np.float32)
    xg = x.reshape(B, T, G, I).transpose(2, 3, 1, 0)  # [G, I, T, B]
    in_maps = []
    for g in range(G):
        xT = np.empty((IP1, T * B), dtype=np.float32)
        xT[:I] = xg[g].reshape(I, T * B)
        xT[I] = 1.0
        wih = np.asarray(Wih[g], dtype=np.float32)  # [3H, I]
        whh = np.asarray(Whh[g], dtype=np.float32)  # [3H, H]
        bi = np.asarray(bih[g], dtype=np.float32)  # [3H]
        bh = np.asarray(bhh[g], dtype=np.float32)  # [3H]
        w_xp_rz = np.empty((IP1, 2 * H), dtype=np.float32)
        w_xp_rz[:I] = wih[: 2 * H].T
        w_xp_rz[I] = bi[: 2 * H]
        w_xp_n = np.empty((IP1, H), dtype=np.float32)
        w_xp_n[:I] = wih[2 * H :].T
        w_xp_n[I] = bi[2 * H :]
        in_maps.append(
            {
                "xT": xT,
                "w_xp_rz": w_xp_rz,
                "w_xp_n": w_xp_n,
                "w_hh_rz": np.ascontiguousarray(whh[: 2 * H].T),
                "w_hh_n": np.ascontiguousarray(whh[2 * H :].T),
                "bhh_r": np.ascontiguousarray(bh[:H, None]),
                "bhh_z": np.ascontiguousarray(bh[H : 2 * H, None]),
                "bhh_n": np.ascontiguousarray(bh[None, 2 * H :]),
                "h0T": np.ascontiguousarray(np.asarray(h0[g], dtype=np.float32).T),
            }
        )
    return in_maps


_NC = None


def _get_program():
    global _NC
    if _NC is None:
        _NC = _build_program()
    return _NC


def run(inputs, trace=False, tmpdir=None):
    """Run on 8 cores; returns ((output, h_final), BassKernelResults)."""
    nc = _get_program()
    in_maps = _prep_inputs(
        inputs["x"], inputs["h0"], inputs["Wih"], inputs["Whh"],
        inputs["bih"], inputs["bhh"],
    )
    res = run_bass_kernel_spmd(
        nc, in_maps, core_ids=list(range(G)), trace=trace, tmpdir=tmpdir
    )
    output = np.empty((B, T, G * H), dtype=np.float32)
    h_final = np.empty((G, B, H), dtype=np.float32)
    for g in range(G):
        yg = res.results[g]["y"].reshape(H, T, B)
        output[:, :, g * H : (g + 1) * H] = yg.transpose(2, 1, 0)
        h_final[g] = yg[:, T - 1, :].T
    return (output, h_final), res


def kernel(**inputs):
    out, _ = run(inputs, trace=False)
    return out
